# revision 4
# baseline (speedup 1.0000x reference)
"""Trainium2 Bass kernel for nn_AttentionBlock_27350351740953.

Module: GroupNorm(32 groups) -> 1x1 qkv conv -> 8-head attention over T=1024
        -> 1x1 proj conv -> residual.   Input x: [8, 512, 32, 32] fp32.

Sharding: data-parallel over batch B=8 -> exactly one batch image per
NeuronCore (8 cores), no collectives.  Weights broadcast to every core.

Per-core layout (c = channel, o = c // 128 tile index, p = c % 128):
  x_sb   [128, 4, 1024] fp32     c-on-partition view of x[b]
  xn_sb  [128, 4, 1024] bf16     groupnormed x
  qk_sb  [128, 8, 1024] bf16     q (o-tiles 0..3, pre-scaled by 1/8) and k
  vT_sb  [128, 8t, 8h, 65] bf16  v transposed per head + ones column (for
                                 softmax denominator via matmul)
  attention per head pair (heads 2i at partitions 0:64, 2i+1 at 64:128):
    S^T = k^T q  (PE, K=64 row-pair-tiled)  -> exp on ACT -> P^T bf16
    h'  = v'^T.T @ P^T  (PE, K=128)  rows 0:64 = unnormalized h, row 64 = l
    h   = h' * (1/l)  (DVE + PE broadcast)
  proj with bias folded in as a K=1 matmul, residual add on DVE.
"""

import os
import sys

for _p in ("/opt/trn_rl_repo",):
    if _p not in sys.path and os.path.isdir(_p):
        sys.path.insert(0, _p)

from contextlib import ExitStack

import numpy as np
import ml_dtypes

import concourse.bass as bass
import concourse.bacc as bacc
import concourse.tile as tile
import concourse.mybir as mybir

B, C, T = 8, 512, 1024
NH, CH = 8, 64          # heads, channels per head
NG, GS = 32, 16         # groups, channels per group
EPS = 1e-5
P = 128
CT = C // P             # 4 channel tiles
TT = T // P             # 8 token tiles
F32 = mybir.dt.float32
BF16 = mybir.dt.bfloat16
AF = mybir.ActivationFunctionType
ALU = mybir.AluOpType

# division strategy for h = h_unnorm / l  (see _divide)
DIV_MODE = os.environ.get("ATTN_DIV_MODE", "shift_pe")


def build_nc():
    """Build the per-core Bass module (same NEFF on all 8 cores)."""
    nc = bacc.Bacc("TRN2", target_bir_lowering=False, debug=False)
    d = {
        "x0": nc.dram_tensor("x0", [C, T], F32, kind="ExternalInput"),
        "gnw": nc.dram_tensor("gnw", [C], F32, kind="ExternalInput"),
        "gnb": nc.dram_tensor("gnb", [C], F32, kind="ExternalInput"),
        "wqkv": nc.dram_tensor("wqkv", [C, 3 * C], BF16, kind="ExternalInput"),
        "bqkv": nc.dram_tensor("bqkv", [3 * C], F32, kind="ExternalInput"),
        "wproj": nc.dram_tensor("wproj", [C, C], BF16, kind="ExternalInput"),
        "bproj": nc.dram_tensor("bproj", [C], F32, kind="ExternalInput"),
        "gmat": nc.dram_tensor("gmat", [P, 8], F32, kind="ExternalInput"),
        "gmatt": nc.dram_tensor("gmatt", [8, P], F32, kind="ExternalInput"),
        "out0": nc.dram_tensor("out0", [C, T], F32, kind="ExternalOutput"),
    }
    with tile.TileContext(nc) as tc:
        with ExitStack() as ctx:
            _emit(ctx, tc, d)
    nc.compile()
    return nc


def _emit(ctx, tc, d):
    nc = tc.nc
    sing = ctx.enter_context(tc.tile_pool(name="sing", bufs=1))
    small = ctx.enter_context(tc.tile_pool(name="small", bufs=2))
    pT = ctx.enter_context(tc.tile_pool(name="pT", bufs=6))
    psS = ctx.enter_context(tc.tile_pool(name="psS", bufs=2, space="PSUM"))
    psH = ctx.enter_context(tc.tile_pool(name="psH", bufs=2, space="PSUM"))

    # ---- persistent SBUF ----
    x_sb = sing.tile([P, CT, T], F32)
    xn_sb = sing.tile([P, CT, T], BF16)
    wqkv_sb = sing.tile([P, CT, 3 * C], BF16)
    qk_sb = sing.tile([P, 8, T], BF16)
    vT_sb = sing.tile([P, TT, NH, CH + 1], BF16)
    h_sb = sing.tile([P, CT, T], BF16)
    wproj_sb = sing.tile([P, CT, C], BF16)
    out_sb = sing.tile([P, CT, T], F32)
    gnw_sb = sing.tile([P, CT], F32)
    gnb_sb = sing.tile([P, CT], F32)
    bqk_sb = sing.tile([P, 8], F32)
    bv_sb = sing.tile([1, C], F32)
    bpr_sb = sing.tile([1, C], F32)
    ones_sb = sing.tile([1, T], F32)
    g_sb = sing.tile([P, 8], F32)
    gt_sb = sing.tile([8, P], F32)
    eps_sb = sing.tile([P, 1], F32)

    # ---- input DMA (split for queue parallelism) ----
    x_r = d["x0"][:].rearrange("(o p) t -> p o t", p=P)
    for o in range(CT):
        nc.sync.dma_start(x_sb[:, o, :], x_r[:, o, :])
    nc.sync.dma_start(gnw_sb, d["gnw"][:].rearrange("(o p) -> p o", p=P))
    nc.sync.dma_start(gnb_sb, d["gnb"][:].rearrange("(o p) -> p o", p=P))
    nc.sync.dma_start(g_sb, d["gmat"][:])
    nc.sync.dma_start(gt_sb, d["gmatt"][:])
    wq_r = d["wqkv"][:].rearrange("(o p) q -> p o q", p=P)
    for o in range(CT):
        nc.sync.dma_start(wqkv_sb[:, o, :], wq_r[:, o, :])
    nc.sync.dma_start(
        bqk_sb, d["bqkv"][:].rearrange("(o p) -> p o", p=P)[:, 0:8]
    )
    nc.sync.dma_start(bv_sb, d["bqkv"][:].rearrange("(a f) -> a f", a=1)[:, 2 * C :])
    nc.sync.dma_start(bpr_sb, d["bproj"][:].rearrange("(a f) -> a f", a=1))
    wp_r = d["wproj"][:].rearrange("(o p) q -> p o q", p=P)
    for o in range(CT):
        nc.sync.dma_start(wproj_sb[:, o, :], wp_r[:, o, :])

    nc.vector.memset(eps_sb, EPS)
    nc.vector.memset(ones_sb, 1.0)
    nc.vector.memset(vT_sb[:, :, :, CH : CH + 1], 1.0)

    # =======================  GroupNorm  =======================
    # per-channel mean/var over T via bn_stats, then 16-channel group
    # aggregation via a tiny indicator matmul, broadcast back the same way.
    mv = sing.tile([P, CT, 2], F32)
    for o in range(CT):
        stats = small.tile([P, 2, 6], F32, tag="bnstats")
        xo = x_sb[:, o, :].rearrange("p (s f) -> p s f", f=512)
        for s in range(2):
            nc.vector.bn_stats(out=stats[:, s, :], in_=xo[:, s, :])
        nc.vector.bn_aggr(out=mv[:, o, :], in_=stats)
    # mv[:,:,1] := E[x^2] per channel = var + mean^2
    msq = small.tile([P, CT], F32, tag="msq")
    nc.vector.tensor_mul(msq, mv[:, :, 0], mv[:, :, 0])
    nc.vector.tensor_add(mv[:, :, 1], mv[:, :, 1], msq)
    # group sums over the 16 channels of each group (partition reduce by G)
    psG = psH.tile([8, CT * 2], F32, tag="h")
    nc.tensor.matmul(
        psG, lhsT=g_sb, rhs=mv.rearrange("p a b -> p (a b)"), start=True, stop=True
    )
    gs = small.tile([8, CT, 2], F32, tag="gs")
    nc.vector.tensor_scalar_mul(
        gs.rearrange("p a b -> p (a b)"), psG, 1.0 / GS
    )
    gmsq = small.tile([8, CT], F32, tag="gmsq")
    nc.vector.tensor_mul(gmsq, gs[:, :, 0], gs[:, :, 0])
    nc.vector.tensor_tensor(gs[:, :, 1], gs[:, :, 1], gmsq, op=ALU.subtract)
    # broadcast (mean_g, var_g) back to channels
    psBC = psH.tile([P, CT * 2], F32, tag="h")
    nc.tensor.matmul(
        psBC, lhsT=gt_sb, rhs=gs.rearrange("p a b -> p (a b)"), start=True, stop=True
    )
    psBCv = psBC.rearrange("p (a b) -> p a b", b=2)
    sc_sb = small.tile([P, CT], F32, tag="sc")
    nc.scalar.activation(
        out=sc_sb, in_=psBCv[:, :, 1], func=AF.Sqrt, bias=eps_sb, scale=1.0
    )
    nc.vector.reciprocal(out=sc_sb, in_=sc_sb)
    nc.vector.tensor_mul(sc_sb, sc_sb, gnw_sb)  # s = rstd * w
    tb_sb = small.tile([P, CT], F32, tag="tb")
    nc.vector.tensor_mul(tb_sb, psBCv[:, :, 0], sc_sb)
    nc.vector.tensor_tensor(tb_sb, gnb_sb, tb_sb, op=ALU.subtract)  # t = b - mean*s
    for o in range(CT):
        nc.vector.tensor_scalar(
            out=xn_sb[:, o, :],
            in0=x_sb[:, o, :],
            scalar1=sc_sb[:, o : o + 1],
            scalar2=tb_sb[:, o : o + 1],
            op0=ALU.mult,
            op1=ALU.add,
        )

    # =======================  qkv matmul  =======================
    # q, k: [o_tile 128, t] psum; copyback adds bias (q weights+bias are
    # pre-scaled by 1/8 on the host so S = k^T q needs no extra scale).
    for j in range(8):
        ps = psS.tile([P, T], F32, tag="s")
        for n in range(2):
            ns = slice(n * 512, (n + 1) * 512)
            for kt in range(CT):
                nc.tensor.matmul(
                    ps[:, ns],
                    lhsT=wqkv_sb[:, kt, j * P : (j + 1) * P],
                    rhs=xn_sb[:, kt, ns],
                    start=(kt == 0),
                    stop=(kt == CT - 1),
                )
        nc.vector.tensor_scalar_add(
            out=qk_sb[:, j, :], in0=ps, scalar1=bqk_sb[:, j : j + 1]
        )
    # v^T: [t_tile 128, o_v 512] = xn^T @ Wv^T, bias folded in as K=1 matmul
    for i in range(TT):
        ps = psH.tile([P, 512], F32, tag="h")
        nc.tensor.matmul(
            ps,
            lhsT=ones_sb[:, i * P : (i + 1) * P],
            rhs=bv_sb,
            start=True,
            stop=False,
        )
        for kt in range(CT):
            nc.tensor.matmul(
                ps,
                lhsT=xn_sb[:, kt, i * P : (i + 1) * P],
                rhs=wqkv_sb[:, kt, 2 * C : 3 * C],
                start=False,
                stop=(kt == CT - 1),
            )
        nc.vector.tensor_copy(
            out=vT_sb[:, i, :, 0:CH],
            in_=ps.rearrange("p (h c) -> p h c", h=NH),
        )

    # =======================  attention  =======================
    # head pairs share the PE array: head 2i on rows 0:64, head 2i+1 on 64:128
    for hp in range(4):
        hA, hB = 2 * hp, 2 * hp + 1
        phA = psH.tile([CH + 1, T], F32, tag="h")
        phB = psH.tile([CH + 1, T], F32, tag="h")
        for si in range(TT):
            sA = psS.tile([P, T], F32, tag="s")
            sB = psS.tile([P, T], F32, tag="s")
            tsl = slice(si * P, (si + 1) * P)
            for n in range(2):
                ns = slice(n * 512, (n + 1) * 512)
                nc.tensor.matmul(
                    sA[:, ns],
                    lhsT=qk_sb[0:CH, 4 + hp, tsl],
                    rhs=qk_sb[0:CH, hp, ns],
                    start=True,
                    stop=True,
                )
                nc.tensor.matmul(
                    sB[:, ns],
                    lhsT=qk_sb[CH:P, 4 + hp, tsl],
                    rhs=qk_sb[CH:P, hp, ns],
                    start=True,
                    stop=True,
                )
            pa = pT.tile([P, T], BF16, tag="pt")
            pb = pT.tile([P, T], BF16, tag="pt")
            nc.scalar.activation(out=pa, in_=sA, func=AF.Exp)
            nc.scalar.activation(out=pb, in_=sB, func=AF.Exp)
            for n in range(2):
                ns = slice(n * 512, (n + 1) * 512)
                nc.tensor.matmul(
                    phA[:, ns],
                    lhsT=vT_sb[:, si, hA, :],
                    rhs=pa[:, ns],
                    start=(si == 0),
                    stop=(si == TT - 1),
                )
                nc.tensor.matmul(
                    phB[:, ns],
                    lhsT=vT_sb[:, si, hB, :],
                    rhs=pb[:, ns],
                    start=(si == 0),
                    stop=(si == TT - 1),
                )
        for hh, ph in ((hA, phA), (hB, phB)):
            _divide(nc, small, psS, ones_sb, h_sb, ph, hh)

    # =======================  proj + residual  =======================
    for j in range(CT):
        ps = psS.tile([P, T], F32, tag="s")
        for n in range(2):
            ns = slice(n * 512, (n + 1) * 512)
            nc.tensor.matmul(
                ps[:, ns],
                lhsT=bpr_sb[:, j * P : (j + 1) * P],
                rhs=ones_sb[:, ns],
                start=True,
                stop=False,
            )
            for kt in range(CT):
                nc.tensor.matmul(
                    ps[:, ns],
                    lhsT=wproj_sb[:, kt, j * P : (j + 1) * P],
                    rhs=h_sb[:, kt, ns],
                    start=False,
                    stop=(kt == CT - 1),
                )
        nc.vector.tensor_add(out=out_sb[:, j, :], in0=ps, in1=x_sb[:, j, :])
    out_r = d["out0"][:].rearrange("(o p) t -> p o t", p=P)
    for j in range(CT):
        nc.sync.dma_start(out_r[:, j, :], out_sb[:, j, :])


def _divide(nc, small, psS, ones_sb, h_sb, ph, hh):
    """h_sb[head slice] = ph[0:64] / ph[64]  (l broadcast along partitions)."""
    kt, pb0 = hh // 2, (hh % 2) * CH
    dst = h_sb[pb0 : pb0 + CH, kt, :]
    if DIV_MODE == "tt_div":
        lrow = ph[CH : CH + 1, :]
        lbc = bass.AP(
            tensor=lrow.tensor, offset=lrow.offset, ap=[[0, CH]] + lrow.ap[1:]
        )
        nc.vector.tensor_tensor(dst, ph[0:CH, :], lbc, op=ALU.divide)
    elif DIV_MODE == "shift_pe":
        r0 = small.tile([1, T], F32, tag="r0")
        nc.vector.reciprocal(out=r0, in_=ph[CH : CH + 1, :])
        rbc = small.tile([CH, T], F32, tag="rbc")
        nc.gpsimd.partition_broadcast(rbc, r0)
        nc.vector.tensor_mul(dst, ph[0:CH, :], rbc)
    else:
        raise ValueError(DIV_MODE)


# =======================  host wrapper  =======================

_G = np.zeros((P, 8), np.float32)
_G[np.arange(P), np.arange(P) // GS] = 1.0


def _prep(x, gn_w, gn_b, qkv_w, qkv_b, proj_w, proj_b):
    xr = np.ascontiguousarray(np.asarray(x, np.float32).reshape(B, C, T))
    wqkv_t = np.asarray(qkv_w, np.float32).T.copy()  # [c, o]
    bqkv = np.asarray(qkv_b, np.float32).copy()
    wqkv_t[:, :C] *= 0.125  # fold softmax scale (1/sqrt(ch)) into q
    bqkv[:C] *= 0.125
    wproj_t = np.asarray(proj_w, np.float32).T.copy()
    shared = {
        "gnw": np.asarray(gn_w, np.float32),
        "gnb": np.asarray(gn_b, np.float32),
        "wqkv": np.ascontiguousarray(wqkv_t.astype(ml_dtypes.bfloat16)),
        "bqkv": bqkv,
        "wproj": np.ascontiguousarray(wproj_t.astype(ml_dtypes.bfloat16)),
        "bproj": np.asarray(proj_b, np.float32),
        "gmat": _G,
        "gmatt": np.ascontiguousarray(_G.T),
    }
    return xr, shared


def kernel(x, gn_w, gn_b, qkv_w, qkv_b, proj_w, proj_b):
    from concourse import bass_utils

    xr, shared = _prep(x, gn_w, gn_b, qkv_w, qkv_b, proj_w, proj_b)
    nc = build_nc()
    in_maps = [{**shared, "x0": xr[b]} for b in range(B)]
    res = bass_utils.run_bass_kernel_spmd(nc, in_maps, core_ids=list(range(B)))
    out = np.stack([r["out0"] for r in res.results], axis=0)
    return out.reshape(B, C, 32, 32).astype(np.float32)
